# revision 11
# baseline (speedup 1.0000x reference)
"""Trainium2 Bass kernel for nn_MultiHeadMALAAttention (head-sharded, v3).

Core c = (batch b = c//2, head-group g = c%2): all N tokens, 4 heads (128
channels).  Stats are head-local -> no collective; host sums the two
partial outputs per batch.

v3 changes vs v2 (driven by measured per-op costs):
- elu+1 via  min(exp(x), max(x+1, 1))  with the +1 folded into the
  projection PSUM by a ones-row matmul, so per tensor it is one ACT
  Exp(bias=-1) + one DVE stt  (was 2 ACT + 1 DVE + extra).
- DMA xbar transposes batched: kc/ksw staged into persistent tiles,
  one transpose per half-tensor (6 instructions total, was 48).
- o-projection moved to stage 2 (x re-streamed), evac to a small work
  tile; no persistent o1p.
- kc/t1 on DVE (2x bf16 mode), ksw/t2 on GPSIMD; x DMA-in batched
  4 chunks at a time; output written as bf16 partials (host sums f32).
"""

import os
import sys

sys.path.insert(0, "/opt/trn_rl_repo")

import numpy as np
import ml_dtypes

B, N, DIM, H, HD = 4, 8192, 256, 8, 32
INTERNAL = H * HD
SCALE = HD ** -0.5
NCORES = 8
CH = 512
NCH = N // CH        # 16
XB = 4 * CH          # x DMA batch = 4 chunks
NXB = N // XB        # 4
KSC = SCALE / N
NBLK = N // 128      # 64

BF16 = ml_dtypes.bfloat16


def _host_consts():
    R = np.zeros((128, 128), np.float32)
    for i in range(64):
        R[2 * i + 1, 2 * i] = -1.0
        R[2 * i, 2 * i + 1] = 1.0
    hmask = np.zeros((128, 128), np.float32)
    for hh in range(4):
        hmask[32 * hh:32 * (hh + 1), 32 * hh:32 * (hh + 1)] = 1.0
    return R, hmask


def _host_prep(x, sin, cos, W_qkvo, b_qkvo, W_lepe, b_lepe, W_proj, b_proj):
    WT = W_qkvo.T.astype(np.float32)
    wp_full = W_proj.T.astype(np.float32)
    wl = W_lepe[:, 0, :].astype(np.float32)
    sinf = np.asarray(sin, np.float32)
    cosf = np.asarray(cos, np.float32)
    ssw = sinf[:, [d ^ 1 for d in range(HD)]]

    R, hmask = _host_consts()
    Rt = np.ascontiguousarray(R.T)
    hmaskS = (hmask * KSC).astype(BF16)
    hmaskM = (hmask * (-KSC / N)).astype(BF16)
    id16 = np.eye(128, dtype=np.float32).astype(BF16)

    cosr = np.ascontiguousarray(np.tile(cosf.T, (4, 1))).astype(BF16)
    sswr = np.ascontiguousarray(np.tile(ssw.T, (4, 1))).astype(BF16)

    use_bias = bool(np.any(b_qkvo) or np.any(b_lepe))

    per_core = []
    for b in range(B):
        xg = np.ascontiguousarray(np.asarray(x[b], np.float32).T).astype(BF16)
        for g in range(2):
            ch = slice(128 * g, 128 * (g + 1))
            wqkvo = np.ascontiguousarray(np.hstack([
                WT[:, 0:256][:, ch], WT[:, 256:512][:, ch],
                WT[:, 512:768][:, ch], WT[:, 768:1024][:, ch]])).astype(BF16)
            wp = np.ascontiguousarray(wp_full[ch, :]).astype(BF16)
            dcw = np.zeros((128, 3, 128), np.float32)
            for tap in range(3):
                np.fill_diagonal(dcw[:, tap, :], wl[ch, tap])
            dcw = np.ascontiguousarray(dcw.reshape(128, 384)).astype(BF16)

            d = {
                "xg": xg, "cosr": cosr, "sswr": sswr,
                "wqkvo": wqkvo, "wp": wp, "dcw": dcw,
                "rblk": R.astype(BF16), "rt": Rt.astype(BF16),
                "hmaskS": hmaskS, "hmaskM": hmaskM, "id16": id16,
            }
            if use_bias:
                bq = np.hstack([
                    np.asarray(b_qkvo[0:256], np.float32)[ch],
                    np.asarray(b_qkvo[256:512], np.float32)[ch],
                    np.asarray(b_qkvo[512:768], np.float32)[ch],
                    np.asarray(b_qkvo[768:1024], np.float32)[ch],
                ]).reshape(1, 512).astype(BF16)
                d["bq"] = bq
                d["blep"] = np.diag(
                    np.asarray(b_lepe, np.float32)[ch]).astype(BF16)
            per_core.append(d)
    return per_core, use_bias


def _build_nc(use_bias: bool):
    from concourse import bacc
    import concourse.mybir as mybir
    import concourse.tile as tile

    dt = mybir.dt
    AF = mybir.ActivationFunctionType
    OP = mybir.AluOpType

    nc = bacc.Bacc(None, target_bir_lowering=False)

    xg_d = nc.dram_tensor("xg", [256, N], dt.bfloat16, kind="ExternalInput")
    cosr_d = nc.dram_tensor("cosr", [128, N], dt.bfloat16, kind="ExternalInput")
    sswr_d = nc.dram_tensor("sswr", [128, N], dt.bfloat16, kind="ExternalInput")
    wqkvo_d = nc.dram_tensor("wqkvo", [256, 512], dt.bfloat16, kind="ExternalInput")
    wp_d = nc.dram_tensor("wp", [128, 256], dt.bfloat16, kind="ExternalInput")
    dcw_d = nc.dram_tensor("dcw", [128, 384], dt.bfloat16, kind="ExternalInput")
    rblk_d = nc.dram_tensor("rblk", [128, 128], dt.bfloat16, kind="ExternalInput")
    rt_d = nc.dram_tensor("rt", [128, 128], dt.bfloat16, kind="ExternalInput")
    hmS_d = nc.dram_tensor("hmaskS", [128, 128], dt.bfloat16, kind="ExternalInput")
    hmM_d = nc.dram_tensor("hmaskM", [128, 128], dt.bfloat16, kind="ExternalInput")
    id16_d = nc.dram_tensor("id16", [128, 128], dt.bfloat16, kind="ExternalInput")
    if use_bias:
        bq_d = nc.dram_tensor("bq", [1, 512], dt.bfloat16, kind="ExternalInput")
        blep_d = nc.dram_tensor("blep", [128, 128], dt.bfloat16,
                                kind="ExternalInput")
    out_d = nc.dram_tensor("out", [N, 256], dt.bfloat16, kind="ExternalOutput")

    with tile.TileContext(nc) as tc:
        with (
            tc.tile_pool(name="const", bufs=1) as const,
            tc.tile_pool(name="work", bufs=2) as work,
        ):
            def load(tname, dten, shape, dtype=dt.bfloat16):
                t_ = const.tile(shape, dtype, tag=tname, name=tname)
                nc.sync.dma_start(out=t_, in_=dten[:, :])
                return t_

            cosr = const.tile([128, N], dt.bfloat16, tag="cosr", name="cosr")
            sswr = const.tile([128, N], dt.bfloat16, tag="sswr", name="sswr")
            for hh in range(2):
                sl = slice(hh * (N // 2), (hh + 1) * (N // 2))
                nc.sync.dma_start(out=cosr[:, sl], in_=cosr_d[:, sl])
                nc.sync.dma_start(out=sswr[:, sl], in_=sswr_d[:, sl])
            wqk = [const.tile([128, 512], dt.bfloat16, tag=f"wqk{k}",
                              name=f"wqk{k}") for k in range(2)]
            for k in range(2):
                nc.sync.dma_start(out=wqk[k], in_=wqkvo_d[128 * k:128 * (k + 1), :])
            wp = load("wp", wp_d, [128, 256])
            dcw = load("dcw", dcw_d, [128, 384])
            rblk = load("rblk", rblk_d, [128, 128])
            rt = load("rt", rt_d, [128, 128])
            hmaskS = load("hmaskS", hmS_d, [128, 128])
            hmaskM = load("hmaskM", hmM_d, [128, 128])
            id16 = load("id16", id16_d, [128, 128])
            onesr = const.tile([1, 128], dt.bfloat16, tag="onesr", name="onesr")
            nc.vector.memset(onesr, 1.0)
            negone = const.tile([128, 1], dt.float32, tag="negone", name="negone")
            nc.vector.memset(negone, -1.0)
            ones5 = const.tile([1, 512], dt.bfloat16, tag="ones5", name="ones5")
            nc.vector.memset(ones5, 1.0)
            inv128 = const.tile([128, 128], dt.bfloat16, tag="inv128",
                                name="inv128")
            nc.vector.memset(inv128, 1.0 / 128.0)
            onesc5 = const.tile([128, 512], dt.bfloat16, tag="onesc5",
                                name="onesc5")
            nc.vector.memset(onesc5, 1.0)
            if use_bias:
                bq = load("bq", bq_d, [1, 512])
                blep = load("blep", blep_d, [128, 128])
                onesc = const.tile([128, CH], dt.bfloat16, tag="onesc",
                                   name="onesc")
                nc.vector.memset(onesc, 1.0)

            q1p = const.tile([128, N], dt.bfloat16, tag="q1p", name="q1p")
            vTp = const.tile([128, N + 2], dt.bfloat16, tag="vTp", name="vTp")
            kcp = const.tile([128, N], dt.bfloat16, tag="kcp", name="kcp")
            kswp = const.tile([128, N], dt.bfloat16, tag="kswp", name="kswp")
            kcT = const.tile([128, N], dt.bfloat16, tag="kcT", name="kcT")
            kswT = const.tile([128, N], dt.bfloat16, tag="kswT", name="kswT")
            vTk = const.tile([128, N], dt.bfloat16, tag="vTk", name="vTk")
            kpart = const.tile([128, NCH], dt.float32, tag="kpart", name="kpart")
            vpart = const.tile([128, NCH], dt.float32, tag="vpart", name="vpart")

            nc.vector.memset(vTp[:, 0:1], 0.0)
            nc.vector.memset(vTp[:, N + 1:N + 2], 0.0)

            def xload(xb):
                bsl = slice(xb * XB, (xb + 1) * XB)
                x0 = work.tile([128, XB], dt.bfloat16, tag="x0", name="x0")
                x1 = work.tile([128, XB], dt.bfloat16, tag="x1", name="x1")
                nc.sync.dma_start(out=x0, in_=xg_d[0:128, bsl])
                nc.sync.dma_start(out=x1, in_=xg_d[128:256, bsl])
                return x0, x1

            # =========================== stage 1 ===========================
            with tc.tile_pool(name="pp1", bufs=2, space="PSUM") as pp1:
                xt = None
                for c in range(NCH):
                    if c % 4 == 0:
                        xt = xload(c // 4)
                    x0 = xt[0][:, (c % 4) * CH:(c % 4 + 1) * CH]
                    x1 = xt[1][:, (c % 4) * CH:(c % 4 + 1) * CH]
                    csl = slice(c * CH, (c + 1) * CH)

                    qk = pp1.tile([128, 2 * CH], dt.float32, tag="qk", name="qk")
                    vps = pp1.tile([128, CH], dt.float32, tag="vps", bufs=3, name="vps")
                    # q' = q+1, k' = k+1 (ones-row bias matmul)
                    nc.tensor.matmul(qk[:, 0:CH], wqk[0][:, 0:128], x0,
                                     start=True, stop=False)
                    nc.tensor.matmul(qk[:, 0:CH], wqk[1][:, 0:128], x1,
                                     start=False, stop=False)
                    nc.tensor.matmul(qk[:, CH:2 * CH], wqk[0][:, 128:256], x0,
                                     start=True, stop=False)
                    nc.tensor.matmul(qk[:, CH:2 * CH], wqk[1][:, 128:256], x1,
                                     start=False, stop=False)
                    if use_bias:
                        nc.tensor.matmul(qk[:, 0:CH], bq[:, 0:128], ones5,
                                         start=False, stop=False)
                        nc.tensor.matmul(qk[:, CH:2 * CH], bq[:, 128:256],
                                         ones5, start=False, stop=False)
                    nc.tensor.matmul(qk[:, 0:CH], inv128, onesc5,
                                     start=False, stop=True)
                    nc.tensor.matmul(qk[:, CH:2 * CH], inv128, onesc5,
                                     start=False, stop=True)
                    nc.tensor.matmul(vps, wqk[0][:, 256:384], x0,
                                     start=True, stop=False)
                    nc.tensor.matmul(vps, wqk[1][:, 256:384], x1,
                                     start=False, stop=not use_bias)
                    if use_bias:
                        nc.tensor.matmul(vps, bq[:, 256:384], ones5,
                                         start=False, stop=True)

                    # q1 = min(exp(q'-1), max(q',1));  same for k1 (+ksum)
                    eq = work.tile([128, CH], dt.bfloat16, tag="eq", bufs=3, name="eq")
                    nc.scalar.activation(eq, qk[:, 0:CH], AF.Exp, bias=negone[:, 0:1])
                    nc.vector.scalar_tensor_tensor(
                        out=q1p[:, csl], in0=qk[:, 0:CH], scalar=1.0, in1=eq,
                        op0=OP.max, op1=OP.min)
                    ek = work.tile([128, CH], dt.bfloat16, tag="ek", bufs=3, name="ek")
                    nc.scalar.activation(ek, qk[:, CH:2 * CH], AF.Exp,
                                         bias=negone[:, 0:1])
                    k1 = work.tile([128, CH], dt.bfloat16, tag="k1", bufs=3, name="k1")
                    nc.vector.scalar_tensor_tensor(
                        out=k1, in0=qk[:, CH:2 * CH], scalar=1.0, in1=ek,
                        op0=OP.max, op1=OP.min, accum_out=kpart[:, c:c + 1])

                    nc.gpsimd.tensor_mul(kcp[:, csl], k1, cosr[:, csl])
                    nc.gpsimd.tensor_mul(kswp[:, csl], k1, sswr[:, csl])

                    nc.scalar.activation(vTp[:, 1 + c * CH:1 + (c + 1) * CH],
                                         vps, AF.Copy,
                                         accum_out=vpart[:, c:c + 1])

                    if c % 4 == 3:
                        qq = c // 4
                        qsl = slice(qq * (N // 4), (qq + 1) * (N // 4))
                        nc.sync.dma_start_transpose(
                            out=kcT[:, qsl].rearrange("p (s x) -> p s x",
                                                      s=N // 512),
                            in_=kcp[:, qsl])
                        nc.sync.dma_start_transpose(
                            out=kswT[:, qsl].rearrange("p (s x) -> p s x",
                                                       s=N // 512),
                            in_=kswp[:, qsl])
                        nc.sync.dma_start_transpose(
                            out=vTk[:, qsl].rearrange("p (s x) -> p s x",
                                                      s=N // 512),
                            in_=vTp[:, 1 + qq * (N // 4):
                                     1 + (qq + 1) * (N // 4)])



            # ====================== stage 1.5: stats =======================
            zblk = const.tile([128, 128], dt.bfloat16, tag="zblk", name="zblk")
            kvblk = const.tile([128, 128], dt.bfloat16, tag="kvblk", name="kvblk")
            kvblk2 = const.tile([128, 128], dt.bfloat16, tag="kvblk2",
                                name="kvblk2")
            mcorr = const.tile([128, 128], dt.bfloat16, tag="mcorr", name="mcorr")
            ksum = const.tile([128, 1], dt.float32, tag="ksum", name="ksum")
            vsum = const.tile([128, 1], dt.float32, tag="vsum", name="vsum")

            with tc.tile_pool(name="ppg", bufs=1, space="PSUM") as ppg:
                nc.vector.tensor_reduce(ksum, kpart[:, 0:NCH],
                                        axis=mybir.AxisListType.X, op=OP.add)
                nc.vector.tensor_tensor(
                    zblk, ksum[:, 0:1].to_broadcast((128, 128)), hmaskS, OP.mult)
                gramC = ppg.tile([128, 128], dt.float32, tag="gramC", name="gramC")
                gramS = ppg.tile([128, 128], dt.float32, tag="gramS", name="gramS")
                for blk in range(NBLK):
                    bsl = slice(blk * 128, (blk + 1) * 128)
                    nc.tensor.matmul(gramC, kcT[:, bsl], vTk[:, bsl],
                                     start=(blk == 0), stop=False)
                    nc.tensor.matmul(gramS, kswT[:, bsl], vTk[:, bsl],
                                     start=(blk == 0), stop=(blk == NBLK - 1))
                gramS_sb = const.tile([128, 128], dt.bfloat16, tag="gramS_sb",
                                      name="gramS_sb")
                nc.vector.tensor_copy(gramS_sb, gramS)
                nc.tensor.matmul(gramC, rblk, gramS_sb, start=False, stop=True)
                nc.vector.tensor_tensor(kvblk, gramC, hmaskS, OP.mult)

                kv2p = ppg.tile([128, 128], dt.float32, tag="kv2p", name="kv2p")
                nc.tensor.matmul(kv2p, rt, kvblk, start=True, stop=True)
                nc.vector.tensor_copy(kvblk2, kv2p)

                nc.vector.tensor_reduce(vsum, vpart[:, 0:NCH],
                                        axis=mybir.AxisListType.X, op=OP.add)
                vs16 = const.tile([128, 1], dt.bfloat16, tag="vs16", name="vs16")
                nc.vector.tensor_copy(vs16, vsum)
                vrp = ppg.tile([128, 128], dt.bfloat16, tag="vrp", name="vrp")
                nc.tensor.transpose(vrp[0:1, 0:128], vs16, id16)
                vrow = const.tile([1, 128], dt.float32, tag="vrow", name="vrow")
                nc.scalar.mul(vrow, vrp[0:1, 0:128], 1.0)
                vrowb = const.tile([128, 128], dt.float32, tag="vrowb",
                                   name="vrowb")
                nc.gpsimd.partition_broadcast(vrowb, vrow)
                tmpM = const.tile([128, 128], dt.bfloat16, tag="tmpM", name="tmpM")
                nc.vector.tensor_tensor(tmpM, vrowb, hmaskM, OP.mult)
                nc.vector.tensor_tensor(
                    mcorr, tmpM, ksum[:, 0:1].to_broadcast((128, 128)), OP.mult)

            # =========================== stage 2 ===========================
            with tc.tile_pool(name="pp2", bufs=2, space="PSUM") as pp2:
                zps = [None] * NCH

                def z_mm(c):
                    zp = pp2.tile([128, CH], dt.float32, tag="zps", name="zps")
                    nc.tensor.matmul(zp, zblk, q1p[:, c * CH:(c + 1) * CH],
                                     start=True, stop=True)
                    return zp

                zps[0] = z_mm(0)
                xt = None
                for c in range(NCH):
                    if c % 4 == 0:
                        xt = xload(c // 4)
                    x0 = xt[0][:, (c % 4) * CH:(c % 4 + 1) * CH]
                    x1 = xt[1][:, (c % 4) * CH:(c % 4 + 1) * CH]
                    csl = slice(c * CH, (c + 1) * CH)
                    if c + 1 < NCH:
                        zps[c + 1] = z_mm(c + 1)

                    ops = pp2.tile([128, CH], dt.float32, tag="ops", name="ops")
                    nc.tensor.matmul(ops, wqk[0][:, 384:512], x0,
                                     start=True, stop=False)
                    nc.tensor.matmul(ops, wqk[1][:, 384:512], x1,
                                     start=False, stop=not use_bias)
                    if use_bias:
                        nc.tensor.matmul(ops, bq[:, 384:512], ones5,
                                         start=False, stop=True)
                    o1 = work.tile([128, CH], dt.bfloat16, tag="o1", bufs=3, name="o1")
                    nc.scalar.activation(o1, ops, AF.Copy)

                    rps = pp2.tile([128, CH], dt.float32, tag="rps", name="rps")
                    for tap in range(3):
                        nc.tensor.matmul(
                            rps, dcw[:, tap * 128:(tap + 1) * 128],
                            vTp[:, c * CH + tap:c * CH + tap + CH],
                            start=(tap == 0), stop=False)
                    if use_bias:
                        nc.tensor.matmul(rps, blep, onesc, start=False,
                                         stop=False)
                    nc.tensor.matmul(rps, mcorr, q1p[:, csl], start=False,
                                     stop=False)

                    rz = work.tile([128, CH], dt.float32, tag="rz", bufs=3, name="rz")
                    nc.vector.reciprocal_approx_fast(out=rz, in_=zps[c])
                    qa = work.tile([128, CH], dt.bfloat16, tag="qa", bufs=3, name="qa")
                    nc.vector.scalar_tensor_tensor(
                        out=qa, in0=rz, scalar=1.0, in1=q1p[:, csl],
                        op0=OP.add, op1=OP.mult)
                    t1 = work.tile([128, CH], dt.bfloat16, tag="t1", bufs=3, name="t1")
                    nc.vector.tensor_mul(t1, qa, cosr[:, csl])
                    t2 = work.tile([128, CH], dt.bfloat16, tag="t2", bufs=3, name="t2")
                    nc.vector.tensor_mul(t2, qa, sswr[:, csl])

                    nc.tensor.matmul(rps, kvblk, t1, start=False, stop=False)
                    nc.tensor.matmul(rps, kvblk2, t2, start=False, stop=True)

                    y = work.tile([128, CH], dt.bfloat16, tag="y", bufs=3, name="y")
                    nc.vector.tensor_mul(y, rps, o1)

                    for half in range(2):
                        outp = pp2.tile([128, 512], dt.float32, tag="outp",
                                        name="outp")
                        for si in range(2):
                            s = half * 2 + si
                            nc.tensor.matmul(outp[:, si * 256:(si + 1) * 256],
                                             y[:, s * 128:(s + 1) * 128], wp,
                                             start=True, stop=True)
                        outsb = work.tile([128, 512], dt.bfloat16, tag="outsb",
                                          name="outsb")
                        nc.scalar.activation(outsb, outp, AF.Copy)
                        dsl = out_d[c * CH + half * 256:c * CH + (half + 1) * 256, :]
                        nc.sync.dma_start(
                            out=dsl.rearrange("(s t) o -> t s o", s=2),
                            in_=outsb)

    nc.compile()
    return nc


_NC_CACHE = {}


def _get_nc(use_bias: bool):
    if use_bias not in _NC_CACHE:
        _NC_CACHE[use_bias] = _build_nc(use_bias)
    return _NC_CACHE[use_bias]


def kernel(x, sin, cos, W_qkvo, b_qkvo, W_lepe, b_lepe, W_proj, b_proj):
    from concourse.bass_utils import run_bass_kernel_spmd
    import concourse.mybir as mybir

    per_core, use_bias = _host_prep(x, sin, cos, W_qkvo, b_qkvo, W_lepe,
                                    b_lepe, W_proj, b_proj)
    nc = _get_nc(use_bias)
    expected = set()
    for alloc in nc.m.functions[0].allocations:
        if isinstance(alloc, mybir.MemoryLocationSet) and alloc.kind == "ExternalInput":
            expected.add(alloc.memorylocations[0].name)
    per_core = [{k: v for k, v in m.items() if k in expected} for m in per_core]
    res = run_bass_kernel_spmd(nc, per_core, core_ids=list(range(NCORES)),
                               trace=bool(os.environ.get("KERNEL_TRACE")))
    if os.environ.get("KERNEL_TRACE"):
        kernel.last_exec_time_ns = res.exec_time_ns
        kernel.last_results = res
    full = np.zeros((B, N, INTERNAL), np.float32)
    for b in range(B):
        full[b] = (res.results[2 * b]["out"].astype(np.float32)
                   + res.results[2 * b + 1]["out"].astype(np.float32))
    full += np.asarray(b_proj, np.float32)[None, None, :]
    return full


# ---------------------------------------------------------- numpy reference

def _numpy_core(d, use_bias, bq=None, blep=None):
    xg = d["xg"].astype(np.float32)
    cosr = d["cosr"].astype(np.float32)
    sswr = d["sswr"].astype(np.float32)
    wqkvo = d["wqkvo"].astype(np.float32)
    wp = d["wp"].astype(np.float32)
    dcw = d["dcw"].astype(np.float32).reshape(128, 3, 128)
    R = d["rblk"].astype(np.float32)
    hmaskS = d["hmaskS"].astype(np.float32)
    hmaskM = d["hmaskM"].astype(np.float32)

    proj = wqkvo.T @ xg
    if use_bias:
        proj = proj + bq.reshape(512, 1).astype(np.float32)
    q, k, v, o = proj[0:128], proj[128:256], proj[256:384], proj[384:512]

    q1 = np.minimum(np.exp(q), np.maximum(q + 1.0, 1.0))
    k1 = np.minimum(np.exp(k), np.maximum(k + 1.0, 1.0))
    ksum = k1.sum(axis=1, keepdims=True)
    vsum = v.sum(axis=1, keepdims=True)

    kc = k1 * cosr
    ksw = k1 * sswr
    gramC = kc @ v.T
    gramS = ksw @ v.T
    kv = (gramC + R.T @ gramS) * hmaskS
    kv2 = R @ kv

    zblk = ksum * hmaskS
    mcorr = (vsum.T * hmaskM) * ksum

    zrep = zblk.T @ q1
    qa = q1 * (1.0 + 1.0 / zrep)
    t1 = qa * cosr
    t2 = qa * sswr

    vpad = np.zeros((128, N + 2), np.float32)
    vpad[:, 1:N + 1] = v
    lepe = np.zeros((128, N), np.float32)
    for tap in range(3):
        lepe += dcw[:, tap, :].T @ vpad[:, tap:tap + N]
    if use_bias:
        lepe += np.diag(blep.astype(np.float32))[:, None]

    rps = kv.T @ t1 + kv2.T @ t2 + mcorr.T @ q1 + lepe
    y = rps * o
    return y.T @ wp


def _numpy_pipeline(per_core, use_bias):
    outs = [
        _numpy_core(d, use_bias, d.get("bq"), d.get("blep"))
        for d in per_core
    ]
    full = np.zeros((B, N, INTERNAL), np.float32)
    for b in range(B):
        full[b] = outs[2 * b] + outs[2 * b + 1]
    return full


if __name__ == "__main__" and os.environ.get("KERNEL_SELFTEST"):
    sys.path.insert(0, os.path.dirname(os.path.abspath(__file__)))
    import reference
    inputs = {k: np.asarray(v) for k, v in reference.setup_inputs().items()}
    expected = np.asarray(reference.reference(**inputs))
    per_core, use_bias = _host_prep(**inputs)
    got = _numpy_pipeline(per_core, use_bias)
    got += np.asarray(inputs["b_proj"], np.float32)[None, None, :]
    rel = np.linalg.norm(got - expected) / np.linalg.norm(expected)
    print("selftest rel err:", rel, "max abs:", np.abs(got - expected).max())

if __name__ == "__main__" and os.environ.get("KERNEL_SIM"):
    sys.path.insert(0, os.path.dirname(os.path.abspath(__file__)))
    from concourse import bass_interp
    import reference
    inputs = {k: np.asarray(v) for k, v in reference.setup_inputs().items()}
    per_core, use_bias = _host_prep(**inputs)
    nc = _get_nc(use_bias)
    import concourse.mybir as mybir
    expected_names = set()
    for alloc in nc.m.functions[0].allocations:
        if isinstance(alloc, mybir.MemoryLocationSet) and alloc.kind == "ExternalInput":
            expected_names.add(alloc.memorylocations[0].name)
    d = per_core[0]
    sim = bass_interp.MultiCoreSim(nc, 1)
    cs = sim.cores[0]
    for name in expected_names:
        if name in d:
            cs.mem_tensor(name)[:] = d[name]
    sim.simulate()
    got = np.asarray(cs.mem_tensor("out"), np.float32)
    want = _numpy_core(d, use_bias, d.get("bq"), d.get("blep"))
    rel = np.linalg.norm(got - want) / np.linalg.norm(want)
    print("sim-vs-numpy rel err:", rel, "max abs:", np.abs(got - want).max())


# revision 13
# speedup vs baseline: 1.0634x; 1.0634x over previous
"""Trainium2 Bass kernel for nn_MultiHeadMALAAttention (head-sharded, v3).

Core c = (batch b = c//2, head-group g = c%2): all N tokens, 4 heads (128
channels).  Stats are head-local -> no collective; host sums the two
partial outputs per batch.

v3 changes vs v2 (driven by measured per-op costs):
- elu+1 via  min(exp(x), max(x+1, 1))  with the +1 folded into the
  projection PSUM by a ones-row matmul, so per tensor it is one ACT
  Exp(bias=-1) + one DVE stt  (was 2 ACT + 1 DVE + extra).
- DMA xbar transposes batched: kc/ksw staged into persistent tiles,
  one transpose per half-tensor (6 instructions total, was 48).
- o-projection moved to stage 2 (x re-streamed), evac to a small work
  tile; no persistent o1p.
- kc/t1 on DVE (2x bf16 mode), ksw/t2 on GPSIMD; x DMA-in batched
  4 chunks at a time; output written as bf16 partials (host sums f32).
"""

import os
import sys

sys.path.insert(0, "/opt/trn_rl_repo")

import numpy as np
import ml_dtypes

B, N, DIM, H, HD = 4, 8192, 256, 8, 32
INTERNAL = H * HD
SCALE = HD ** -0.5
NCORES = 8
CH = 512
NCH = N // CH        # 16
XB = 4 * CH          # x DMA batch = 4 chunks
NXB = N // XB        # 4
KSC = SCALE / N
NBLK = N // 128      # 64

BF16 = ml_dtypes.bfloat16


def _host_consts():
    R = np.zeros((128, 128), np.float32)
    for i in range(64):
        R[2 * i + 1, 2 * i] = -1.0
        R[2 * i, 2 * i + 1] = 1.0
    hmask = np.zeros((128, 128), np.float32)
    for hh in range(4):
        hmask[32 * hh:32 * (hh + 1), 32 * hh:32 * (hh + 1)] = 1.0
    return R, hmask


def _host_prep(x, sin, cos, W_qkvo, b_qkvo, W_lepe, b_lepe, W_proj, b_proj):
    WT = W_qkvo.T.astype(np.float32)
    wp_full = W_proj.T.astype(np.float32)
    wl = W_lepe[:, 0, :].astype(np.float32)
    sinf = np.asarray(sin, np.float32)
    cosf = np.asarray(cos, np.float32)
    ssw = sinf[:, [d ^ 1 for d in range(HD)]]

    R, hmask = _host_consts()
    Rt = np.ascontiguousarray(R.T)
    hmaskS = (hmask * KSC).astype(BF16)
    hmaskM = (hmask * (-KSC / N)).astype(BF16)
    id16 = np.eye(128, dtype=np.float32).astype(BF16)

    cosr = np.ascontiguousarray(np.tile(cosf.T, (4, 1))).astype(BF16)
    sswr = np.ascontiguousarray(np.tile(ssw.T, (4, 1))).astype(BF16)

    use_bias = bool(np.any(b_qkvo) or np.any(b_lepe))

    per_core = []
    for b in range(B):
        xg = np.ascontiguousarray(np.asarray(x[b], np.float32).T).astype(BF16)
        for g in range(2):
            ch = slice(128 * g, 128 * (g + 1))
            wqkvo = np.ascontiguousarray(np.hstack([
                WT[:, 0:256][:, ch], WT[:, 256:512][:, ch],
                WT[:, 512:768][:, ch], WT[:, 768:1024][:, ch]])).astype(BF16)
            wp = np.ascontiguousarray(wp_full[ch, :]).astype(BF16)
            dcw = np.zeros((128, 3, 128), np.float32)
            for tap in range(3):
                np.fill_diagonal(dcw[:, tap, :], wl[ch, tap])
            dcw = np.ascontiguousarray(dcw.reshape(128, 384)).astype(BF16)

            d = {
                "xg": xg, "cosr": cosr, "sswr": sswr,
                "wqkvo": wqkvo, "wp": wp, "dcw": dcw,
                "rblk": R.astype(BF16), "rt": Rt.astype(BF16),
                "hmaskS": hmaskS, "hmaskM": hmaskM, "id16": id16,
            }
            if use_bias:
                bq = np.hstack([
                    np.asarray(b_qkvo[0:256], np.float32)[ch],
                    np.asarray(b_qkvo[256:512], np.float32)[ch],
                    np.asarray(b_qkvo[512:768], np.float32)[ch],
                    np.asarray(b_qkvo[768:1024], np.float32)[ch],
                ]).reshape(1, 512).astype(BF16)
                d["bq"] = bq
                d["blep"] = np.diag(
                    np.asarray(b_lepe, np.float32)[ch]).astype(BF16)
            per_core.append(d)
    return per_core, use_bias


def _build_nc(use_bias: bool):
    from concourse import bacc
    import concourse.mybir as mybir
    import concourse.tile as tile

    dt = mybir.dt
    AF = mybir.ActivationFunctionType
    OP = mybir.AluOpType

    nc = bacc.Bacc(None, target_bir_lowering=False)

    xg_d = nc.dram_tensor("xg", [256, N], dt.bfloat16, kind="ExternalInput")
    cosr_d = nc.dram_tensor("cosr", [128, N], dt.bfloat16, kind="ExternalInput")
    sswr_d = nc.dram_tensor("sswr", [128, N], dt.bfloat16, kind="ExternalInput")
    wqkvo_d = nc.dram_tensor("wqkvo", [256, 512], dt.bfloat16, kind="ExternalInput")
    wp_d = nc.dram_tensor("wp", [128, 256], dt.bfloat16, kind="ExternalInput")
    dcw_d = nc.dram_tensor("dcw", [128, 384], dt.bfloat16, kind="ExternalInput")
    rblk_d = nc.dram_tensor("rblk", [128, 128], dt.bfloat16, kind="ExternalInput")
    rt_d = nc.dram_tensor("rt", [128, 128], dt.bfloat16, kind="ExternalInput")
    hmS_d = nc.dram_tensor("hmaskS", [128, 128], dt.bfloat16, kind="ExternalInput")
    hmM_d = nc.dram_tensor("hmaskM", [128, 128], dt.bfloat16, kind="ExternalInput")
    id16_d = nc.dram_tensor("id16", [128, 128], dt.bfloat16, kind="ExternalInput")
    if use_bias:
        bq_d = nc.dram_tensor("bq", [1, 512], dt.bfloat16, kind="ExternalInput")
        blep_d = nc.dram_tensor("blep", [128, 128], dt.bfloat16,
                                kind="ExternalInput")
    out_d = nc.dram_tensor("out", [N, 256], dt.bfloat16, kind="ExternalOutput")

    with tile.TileContext(nc) as tc:
        with (
            tc.tile_pool(name="const", bufs=1) as const,
            tc.tile_pool(name="work", bufs=2) as work,
        ):
            def load(tname, dten, shape, dtype=dt.bfloat16):
                t_ = const.tile(shape, dtype, tag=tname, name=tname)
                nc.sync.dma_start(out=t_, in_=dten[:, :])
                return t_

            cosr = const.tile([128, N], dt.bfloat16, tag="cosr", name="cosr")
            sswr = const.tile([128, N], dt.bfloat16, tag="sswr", name="sswr")
            for hh in range(2):
                sl = slice(hh * (N // 2), (hh + 1) * (N // 2))
                nc.sync.dma_start(out=cosr[:, sl], in_=cosr_d[:, sl])
                nc.sync.dma_start(out=sswr[:, sl], in_=sswr_d[:, sl])
            wqk = [const.tile([128, 512], dt.bfloat16, tag=f"wqk{k}",
                              name=f"wqk{k}") for k in range(2)]
            for k in range(2):
                nc.sync.dma_start(out=wqk[k], in_=wqkvo_d[128 * k:128 * (k + 1), :])
            wp = load("wp", wp_d, [128, 256])
            dcw = load("dcw", dcw_d, [128, 384])
            rblk = load("rblk", rblk_d, [128, 128])
            rt = load("rt", rt_d, [128, 128])
            hmaskS = load("hmaskS", hmS_d, [128, 128])
            hmaskM = load("hmaskM", hmM_d, [128, 128])
            id16 = load("id16", id16_d, [128, 128])
            onesr = const.tile([1, 128], dt.bfloat16, tag="onesr", name="onesr")
            nc.vector.memset(onesr, 1.0)
            negone = const.tile([128, 1], dt.float32, tag="negone", name="negone")
            nc.vector.memset(negone, -1.0)
            ones5 = const.tile([1, 512], dt.bfloat16, tag="ones5", name="ones5")
            nc.vector.memset(ones5, 1.0)
            inv128 = const.tile([128, 128], dt.bfloat16, tag="inv128",
                                name="inv128")
            nc.vector.memset(inv128, 1.0 / 128.0)
            onesc5 = const.tile([128, 512], dt.bfloat16, tag="onesc5",
                                name="onesc5")
            nc.vector.memset(onesc5, 1.0)
            if use_bias:
                bq = load("bq", bq_d, [1, 512])
                blep = load("blep", blep_d, [128, 128])
                onesc = const.tile([128, CH], dt.bfloat16, tag="onesc",
                                   name="onesc")
                nc.vector.memset(onesc, 1.0)

            q1p = const.tile([128, N], dt.bfloat16, tag="q1p", name="q1p")
            vTp = const.tile([128, N + 2], dt.bfloat16, tag="vTp", name="vTp")
            kcp = const.tile([128, N], dt.bfloat16, tag="kcp", name="kcp")
            kswp = const.tile([128, N], dt.bfloat16, tag="kswp", name="kswp")
            kcT = const.tile([128, N], dt.bfloat16, tag="kcT", name="kcT")
            kswT = const.tile([128, N], dt.bfloat16, tag="kswT", name="kswT")
            vTk = const.tile([128, N], dt.bfloat16, tag="vTk", name="vTk")
            kpart = const.tile([128, NCH], dt.float32, tag="kpart", name="kpart")
            vpart = const.tile([128, NCH], dt.float32, tag="vpart", name="vpart")

            nc.vector.memset(vTp[:, 0:1], 0.0)
            nc.vector.memset(vTp[:, N + 1:N + 2], 0.0)

            def xload(xb):
                bsl = slice(xb * XB, (xb + 1) * XB)
                x0 = work.tile([128, XB], dt.bfloat16, tag="x0", name="x0")
                x1 = work.tile([128, XB], dt.bfloat16, tag="x1", name="x1")
                nc.sync.dma_start(out=x0, in_=xg_d[0:128, bsl])
                nc.sync.dma_start(out=x1, in_=xg_d[128:256, bsl])
                return x0, x1

            # =========================== stage 1 ===========================
            with tc.tile_pool(name="pp1", bufs=2, space="PSUM") as pp1:
                xt = None
                for c in range(NCH):
                    if c % 4 == 0:
                        xt = xload(c // 4)
                    x0 = xt[0][:, (c % 4) * CH:(c % 4 + 1) * CH]
                    x1 = xt[1][:, (c % 4) * CH:(c % 4 + 1) * CH]
                    csl = slice(c * CH, (c + 1) * CH)

                    qk = pp1.tile([128, 2 * CH], dt.float32, tag="qk", name="qk")
                    vps = pp1.tile([128, CH], dt.float32, tag="vps", name="vps")
                    # q' = q+1, k' = k+1 (ones-row bias matmul)
                    nc.tensor.matmul(qk[:, 0:CH], wqk[0][:, 0:128], x0,
                                     start=True, stop=False)
                    nc.tensor.matmul(qk[:, 0:CH], wqk[1][:, 0:128], x1,
                                     start=False, stop=False)
                    nc.tensor.matmul(qk[:, CH:2 * CH], wqk[0][:, 128:256], x0,
                                     start=True, stop=False)
                    nc.tensor.matmul(qk[:, CH:2 * CH], wqk[1][:, 128:256], x1,
                                     start=False, stop=False)
                    if use_bias:
                        nc.tensor.matmul(qk[:, 0:CH], bq[:, 0:128], ones5,
                                         start=False, stop=False)
                        nc.tensor.matmul(qk[:, CH:2 * CH], bq[:, 128:256],
                                         ones5, start=False, stop=False)
                    nc.tensor.matmul(qk[:, 0:CH], inv128, onesc5,
                                     start=False, stop=True)
                    nc.tensor.matmul(qk[:, CH:2 * CH], inv128, onesc5,
                                     start=False, stop=True)
                    nc.tensor.matmul(vps, wqk[0][:, 256:384], x0,
                                     start=True, stop=False)
                    nc.tensor.matmul(vps, wqk[1][:, 256:384], x1,
                                     start=False, stop=not use_bias)
                    if use_bias:
                        nc.tensor.matmul(vps, bq[:, 256:384], ones5,
                                         start=False, stop=True)

                    # q1 = min(exp(q'-1), max(q',1));  same for k1 (+ksum)
                    eq = work.tile([128, CH], dt.bfloat16, tag="eq", name="eq")
                    nc.scalar.activation(eq, qk[:, 0:CH], AF.Exp, bias=negone[:, 0:1])
                    nc.vector.scalar_tensor_tensor(
                        out=q1p[:, csl], in0=qk[:, 0:CH], scalar=1.0, in1=eq,
                        op0=OP.max, op1=OP.min)
                    ek = work.tile([128, CH], dt.bfloat16, tag="ek", name="ek")
                    nc.scalar.activation(ek, qk[:, CH:2 * CH], AF.Exp,
                                         bias=negone[:, 0:1])
                    k1 = work.tile([128, CH], dt.bfloat16, tag="k1", name="k1")
                    nc.vector.scalar_tensor_tensor(
                        out=k1, in0=qk[:, CH:2 * CH], scalar=1.0, in1=ek,
                        op0=OP.max, op1=OP.min, accum_out=kpart[:, c:c + 1])

                    nc.gpsimd.tensor_mul(kcp[:, csl], k1, cosr[:, csl])
                    nc.gpsimd.tensor_mul(kswp[:, csl], k1, sswr[:, csl])

                    nc.scalar.activation(vTp[:, 1 + c * CH:1 + (c + 1) * CH],
                                         vps, AF.Copy,
                                         accum_out=vpart[:, c:c + 1])

                    if c % 4 == 3:
                        qq = c // 4
                        qsl = slice(qq * (N // 4), (qq + 1) * (N // 4))
                        nc.sync.dma_start_transpose(
                            out=kcT[:, qsl].rearrange("p (s x) -> p s x",
                                                      s=N // 512),
                            in_=kcp[:, qsl])
                        nc.sync.dma_start_transpose(
                            out=kswT[:, qsl].rearrange("p (s x) -> p s x",
                                                       s=N // 512),
                            in_=kswp[:, qsl])
                        nc.sync.dma_start_transpose(
                            out=vTk[:, qsl].rearrange("p (s x) -> p s x",
                                                      s=N // 512),
                            in_=vTp[:, 1 + qq * (N // 4):
                                     1 + (qq + 1) * (N // 4)])



            # ====================== stage 1.5: stats =======================
            zblk = const.tile([128, 128], dt.bfloat16, tag="zblk", name="zblk")
            kvblk = const.tile([128, 128], dt.bfloat16, tag="kvblk", name="kvblk")
            kvblk2 = const.tile([128, 128], dt.bfloat16, tag="kvblk2",
                                name="kvblk2")
            mcorr = const.tile([128, 128], dt.bfloat16, tag="mcorr", name="mcorr")
            ksum = const.tile([128, 1], dt.float32, tag="ksum", name="ksum")
            vsum = const.tile([128, 1], dt.float32, tag="vsum", name="vsum")

            with tc.tile_pool(name="ppg", bufs=1, space="PSUM") as ppg:
                nc.vector.tensor_reduce(ksum, kpart[:, 0:NCH],
                                        axis=mybir.AxisListType.X, op=OP.add)
                nc.vector.tensor_tensor(
                    zblk, ksum[:, 0:1].to_broadcast((128, 128)), hmaskS, OP.mult)
                gramC = ppg.tile([128, 128], dt.float32, tag="gramC", name="gramC")
                gramS = ppg.tile([128, 128], dt.float32, tag="gramS", name="gramS")
                for blk in range(NBLK):
                    bsl = slice(blk * 128, (blk + 1) * 128)
                    nc.tensor.matmul(gramC, kcT[:, bsl], vTk[:, bsl],
                                     start=(blk == 0), stop=False)
                    nc.tensor.matmul(gramS, kswT[:, bsl], vTk[:, bsl],
                                     start=(blk == 0), stop=(blk == NBLK - 1))
                gramS_sb = const.tile([128, 128], dt.bfloat16, tag="gramS_sb",
                                      name="gramS_sb")
                nc.vector.tensor_copy(gramS_sb, gramS)
                nc.tensor.matmul(gramC, rblk, gramS_sb, start=False, stop=True)
                nc.vector.tensor_tensor(kvblk, gramC, hmaskS, OP.mult)

                kv2p = ppg.tile([128, 128], dt.float32, tag="kv2p", name="kv2p")
                nc.tensor.matmul(kv2p, rt, kvblk, start=True, stop=True)
                nc.vector.tensor_copy(kvblk2, kv2p)

                nc.vector.tensor_reduce(vsum, vpart[:, 0:NCH],
                                        axis=mybir.AxisListType.X, op=OP.add)
                vs16 = const.tile([128, 1], dt.bfloat16, tag="vs16", name="vs16")
                nc.vector.tensor_copy(vs16, vsum)
                vrp = ppg.tile([128, 128], dt.bfloat16, tag="vrp", name="vrp")
                nc.tensor.transpose(vrp[0:1, 0:128], vs16, id16)
                vrow = const.tile([1, 128], dt.float32, tag="vrow", name="vrow")
                nc.scalar.mul(vrow, vrp[0:1, 0:128], 1.0)
                vrowb = const.tile([128, 128], dt.float32, tag="vrowb",
                                   name="vrowb")
                nc.gpsimd.partition_broadcast(vrowb, vrow)
                tmpM = const.tile([128, 128], dt.bfloat16, tag="tmpM", name="tmpM")
                nc.vector.tensor_tensor(tmpM, vrowb, hmaskM, OP.mult)
                nc.vector.tensor_tensor(
                    mcorr, tmpM, ksum[:, 0:1].to_broadcast((128, 128)), OP.mult)

            # =========================== stage 2 ===========================
            with tc.tile_pool(name="pp2", bufs=2, space="PSUM") as pp2:
                zps = [None] * NCH

                def z_mm(c):
                    zp = pp2.tile([128, CH], dt.float32, tag="zps", name="zps")
                    nc.tensor.matmul(zp, zblk, q1p[:, c * CH:(c + 1) * CH],
                                     start=True, stop=True)
                    return zp

                zps[0] = z_mm(0)
                xt = None
                pend = None

                def finish(p):
                    pc, prps, po1, pt1, pt2 = p
                    nc.tensor.matmul(prps, kvblk, pt1, start=False, stop=False)
                    nc.tensor.matmul(prps, kvblk2, pt2, start=False, stop=True)
                    y = work.tile([128, CH], dt.bfloat16, tag="y", bufs=3,
                                  name="y")
                    nc.vector.tensor_mul(y, prps, po1)
                    for half in range(2):
                        outp = pp2.tile([128, 512], dt.float32, tag="outp",
                                        name="outp")
                        for si in range(2):
                            s = half * 2 + si
                            nc.tensor.matmul(outp[:, si * 256:(si + 1) * 256],
                                             y[:, s * 128:(s + 1) * 128], wp,
                                             start=True, stop=True)
                        outsb = work.tile([128, 512], dt.bfloat16, tag="outsb",
                                          name="outsb")
                        nc.scalar.activation(outsb, outp, AF.Copy)
                        dsl = out_d[pc * CH + half * 256:
                                    pc * CH + (half + 1) * 256, :]
                        nc.sync.dma_start(
                            out=dsl.rearrange("(s t) o -> t s o", s=2),
                            in_=outsb)

                for c in range(NCH):
                    if c % 4 == 0:
                        xt = xload(c // 4)
                    x0 = xt[0][:, (c % 4) * CH:(c % 4 + 1) * CH]
                    x1 = xt[1][:, (c % 4) * CH:(c % 4 + 1) * CH]
                    csl = slice(c * CH, (c + 1) * CH)
                    if c + 1 < NCH:
                        zps[c + 1] = z_mm(c + 1)

                    ops = pp2.tile([128, CH], dt.float32, tag="ops", name="ops")
                    nc.tensor.matmul(ops, wqk[0][:, 384:512], x0,
                                     start=True, stop=False)
                    nc.tensor.matmul(ops, wqk[1][:, 384:512], x1,
                                     start=False, stop=not use_bias)
                    if use_bias:
                        nc.tensor.matmul(ops, bq[:, 384:512], ones5,
                                         start=False, stop=True)
                    o1 = work.tile([128, CH], dt.bfloat16, tag="o1", name="o1")
                    nc.scalar.activation(o1, ops, AF.Copy)

                    rps = pp2.tile([128, CH], dt.float32, tag="rps", name="rps")
                    for tap in range(3):
                        nc.tensor.matmul(
                            rps, dcw[:, tap * 128:(tap + 1) * 128],
                            vTp[:, c * CH + tap:c * CH + tap + CH],
                            start=(tap == 0), stop=False)
                    if use_bias:
                        nc.tensor.matmul(rps, blep, onesc, start=False,
                                         stop=False)
                    nc.tensor.matmul(rps, mcorr, q1p[:, csl], start=False,
                                     stop=False)

                    rz = work.tile([128, CH], dt.float32, tag="rz", bufs=3, name="rz")
                    nc.vector.reciprocal_approx_fast(out=rz, in_=zps[c])
                    qa = work.tile([128, CH], dt.bfloat16, tag="qa", bufs=3, name="qa")
                    nc.vector.scalar_tensor_tensor(
                        out=qa, in0=rz, scalar=1.0, in1=q1p[:, csl],
                        op0=OP.add, op1=OP.mult)
                    t1 = work.tile([128, CH], dt.bfloat16, tag="t1", bufs=3, name="t1")
                    nc.vector.tensor_mul(t1, qa, cosr[:, csl])
                    t2 = work.tile([128, CH], dt.bfloat16, tag="t2", bufs=3, name="t2")
                    nc.vector.tensor_mul(t2, qa, sswr[:, csl])

                    if pend is not None:
                        finish(pend)
                    pend = (c, rps, o1, t1, t2)
                finish(pend)

    nc.compile()
    return nc


_NC_CACHE = {}


def _get_nc(use_bias: bool):
    if use_bias not in _NC_CACHE:
        _NC_CACHE[use_bias] = _build_nc(use_bias)
    return _NC_CACHE[use_bias]


def kernel(x, sin, cos, W_qkvo, b_qkvo, W_lepe, b_lepe, W_proj, b_proj):
    from concourse.bass_utils import run_bass_kernel_spmd
    import concourse.mybir as mybir

    per_core, use_bias = _host_prep(x, sin, cos, W_qkvo, b_qkvo, W_lepe,
                                    b_lepe, W_proj, b_proj)
    nc = _get_nc(use_bias)
    expected = set()
    for alloc in nc.m.functions[0].allocations:
        if isinstance(alloc, mybir.MemoryLocationSet) and alloc.kind == "ExternalInput":
            expected.add(alloc.memorylocations[0].name)
    per_core = [{k: v for k, v in m.items() if k in expected} for m in per_core]
    res = run_bass_kernel_spmd(nc, per_core, core_ids=list(range(NCORES)),
                               trace=bool(os.environ.get("KERNEL_TRACE")))
    if os.environ.get("KERNEL_TRACE"):
        kernel.last_exec_time_ns = res.exec_time_ns
        kernel.last_results = res
    full = np.zeros((B, N, INTERNAL), np.float32)
    for b in range(B):
        full[b] = (res.results[2 * b]["out"].astype(np.float32)
                   + res.results[2 * b + 1]["out"].astype(np.float32))
    full += np.asarray(b_proj, np.float32)[None, None, :]
    return full


# ---------------------------------------------------------- numpy reference

def _numpy_core(d, use_bias, bq=None, blep=None):
    xg = d["xg"].astype(np.float32)
    cosr = d["cosr"].astype(np.float32)
    sswr = d["sswr"].astype(np.float32)
    wqkvo = d["wqkvo"].astype(np.float32)
    wp = d["wp"].astype(np.float32)
    dcw = d["dcw"].astype(np.float32).reshape(128, 3, 128)
    R = d["rblk"].astype(np.float32)
    hmaskS = d["hmaskS"].astype(np.float32)
    hmaskM = d["hmaskM"].astype(np.float32)

    proj = wqkvo.T @ xg
    if use_bias:
        proj = proj + bq.reshape(512, 1).astype(np.float32)
    q, k, v, o = proj[0:128], proj[128:256], proj[256:384], proj[384:512]

    q1 = np.minimum(np.exp(q), np.maximum(q + 1.0, 1.0))
    k1 = np.minimum(np.exp(k), np.maximum(k + 1.0, 1.0))
    ksum = k1.sum(axis=1, keepdims=True)
    vsum = v.sum(axis=1, keepdims=True)

    kc = k1 * cosr
    ksw = k1 * sswr
    gramC = kc @ v.T
    gramS = ksw @ v.T
    kv = (gramC + R.T @ gramS) * hmaskS
    kv2 = R @ kv

    zblk = ksum * hmaskS
    mcorr = (vsum.T * hmaskM) * ksum

    zrep = zblk.T @ q1
    qa = q1 * (1.0 + 1.0 / zrep)
    t1 = qa * cosr
    t2 = qa * sswr

    vpad = np.zeros((128, N + 2), np.float32)
    vpad[:, 1:N + 1] = v
    lepe = np.zeros((128, N), np.float32)
    for tap in range(3):
        lepe += dcw[:, tap, :].T @ vpad[:, tap:tap + N]
    if use_bias:
        lepe += np.diag(blep.astype(np.float32))[:, None]

    rps = kv.T @ t1 + kv2.T @ t2 + mcorr.T @ q1 + lepe
    y = rps * o
    return y.T @ wp


def _numpy_pipeline(per_core, use_bias):
    outs = [
        _numpy_core(d, use_bias, d.get("bq"), d.get("blep"))
        for d in per_core
    ]
    full = np.zeros((B, N, INTERNAL), np.float32)
    for b in range(B):
        full[b] = outs[2 * b] + outs[2 * b + 1]
    return full


if __name__ == "__main__" and os.environ.get("KERNEL_SELFTEST"):
    sys.path.insert(0, os.path.dirname(os.path.abspath(__file__)))
    import reference
    inputs = {k: np.asarray(v) for k, v in reference.setup_inputs().items()}
    expected = np.asarray(reference.reference(**inputs))
    per_core, use_bias = _host_prep(**inputs)
    got = _numpy_pipeline(per_core, use_bias)
    got += np.asarray(inputs["b_proj"], np.float32)[None, None, :]
    rel = np.linalg.norm(got - expected) / np.linalg.norm(expected)
    print("selftest rel err:", rel, "max abs:", np.abs(got - expected).max())

if __name__ == "__main__" and os.environ.get("KERNEL_SIM"):
    sys.path.insert(0, os.path.dirname(os.path.abspath(__file__)))
    from concourse import bass_interp
    import reference
    inputs = {k: np.asarray(v) for k, v in reference.setup_inputs().items()}
    per_core, use_bias = _host_prep(**inputs)
    nc = _get_nc(use_bias)
    import concourse.mybir as mybir
    expected_names = set()
    for alloc in nc.m.functions[0].allocations:
        if isinstance(alloc, mybir.MemoryLocationSet) and alloc.kind == "ExternalInput":
            expected_names.add(alloc.memorylocations[0].name)
    d = per_core[0]
    sim = bass_interp.MultiCoreSim(nc, 1)
    cs = sim.cores[0]
    for name in expected_names:
        if name in d:
            cs.mem_tensor(name)[:] = d[name]
    sim.simulate()
    got = np.asarray(cs.mem_tensor("out"), np.float32)
    want = _numpy_core(d, use_bias, d.get("bq"), d.get("blep"))
    rel = np.linalg.norm(got - want) / np.linalg.norm(want)
    print("sim-vs-numpy rel err:", rel, "max abs:", np.abs(got - want).max())
